# revision 59
# baseline (speedup 1.0000x reference)
"""Trainium2 Bass kernel for nn_Attention_70557722739202.

Standard MHA block: qkv = x @ Wqkv.T + bqkv; attn = softmax(q k^T / 8);
out = (attn v) @ Wproj.T + bproj, with B=4, N=2048, C=768, H=12, hd=64
(ratio == 1 so the slimmable slicing is identity).

Sharding (8 cores): batch x head-group.  Core c handles batch c//2 and
heads [6*(c%2), 6*(c%2)+6).  Wqkv rows / Wproj cols are sharded by head;
each core emits a partial projection output [2048, 768] and the host sums
the two partials per batch (+ bproj).

Per-core dataflow (all PE matmuls in bfloat16, fp32 PSUM; bf16 moving
data streams 1 row/cycle at the PE's 2.4GHz vs 2 rows/cycle-pair for
4-byte fp32r, measured on HW):
  - x.T, W slices DMA'd with input-channel on partitions (bf16 host-cast).
  - q.T/k.T computed channel-major ([128, 2048] tiles, 2 heads per tile);
    v computed seq-major with a fused ones column for softmax row-sums.
    qkv biases are emitted as K=1 rank-1 matmuls only when nonzero.
  - S.T = k q^T per head via row-packed K=64 matmuls (2 heads concurrent
    in the PE array via tile_position row groups).  exp(S/8) runs on
    ScalarE straight out of PSUM with the 1/8 scale folded into the
    activation's free affine; no max subtraction (scores are O(1) by
    construction).  ScalarE exp is the pacing engine: 2 x [128,1024]
    activations per key chunk ~ 2.23us vs ~1.7us of PE work.
  - U.T = [v | 1]^T expS.T accumulated over key chunks -> rows 0..63 are
    the unnormalized attention output, row 64 the softmax denominator.
  - normalize: DVE reciprocal_approx_fast + GpSimd partition_broadcast +
    DVE multiply (head B through a bf16 staging tile + DMA partition
    shift).
  - projection: attnT chunks (stationary) x Wproj.T slices, PSUM K-accum.

Scheduling: software-pipelined per (pair, query-half): the U matmuls of
key chunk i are emitted after chunk i+1's S matmuls + exp; the exp
output pool is 6 deep so the Scalar engine can run a chunk ahead instead
of ping-ponging with the PE (with 2 bufs each exp waits for the U of the
previous chunk to release its slot - measured 1.1-1.3us stalls).  QKV for
pair t+1 and the first half of the projection are emitted as PE filler
inside pair t's attention.  Denser variants (continuous cross-boundary
pipeline with U deferred by 4 chunks) measured SLOWER: keeping the PE
gap-free saturates shared SBUF/PSUM bandwidth and inflates every
engine's per-instruction time by ~1.2x.
Measured on HW: fp32r baseline 429us -> bf16 369us -> this schedule 330us.
"""

import os
import sys

for _p in ("/opt/trn_rl_repo",):
    if os.path.isdir(_p) and _p not in sys.path:
        sys.path.insert(0, _p)

import numpy as np

import concourse.bacc as bacc
import concourse.mybir as mybir
import concourse.tile as tile
from concourse.bass_utils import run_bass_kernel_spmd

DIM = 768
NHEADS = 12
B, N = 4, 2048
HD = 64          # head dim
NCORES = 8
HPC = 6          # heads per core
PAIRS = 3        # head pairs per core
GPB = 2          # head groups per batch
CH = HPC * HD    # 384 output channels per core
SCALE = (DIM // NHEADS) ** -0.5
P = 128
QT = 1024        # query tile width (PSUM: 2 banks per S tile)
NKC = N // P     # 16 key chunks
KC = DIM // P    # 6 input-channel chunks
F32 = mybir.dt.float32
BF16 = mybir.dt.bfloat16
EXP = mybir.ActivationFunctionType.Exp

_PROGRAMS = {}


def _emit(tc, xT_d, wqkT_d, wvT_d, bqk_d, bv_d, wpT_d, y_d, with_bias=True):
    nc = tc.nc

    from contextlib import ExitStack

    with ExitStack() as ctx:
        const = ctx.enter_context(tc.tile_pool(name="const", bufs=1))
        qkpool = ctx.enter_context(tc.tile_pool(name="qkpool", bufs=4))
        atpool = ctx.enter_context(tc.tile_pool(name="atpool", bufs=3))
        # epool depth is the scalar-engine runahead: with only 2 bufs the
        # exp of chunk i+1 waits for U of chunk i to finish reading, so the
        # Scalar and Tensor engines ping-pong instead of streaming.  6 bufs
        # = 3 chunks in flight.
        epool = ctx.enter_context(tc.tile_pool(name="epool", bufs=6))
        rpool = ctx.enter_context(tc.tile_pool(name="rpool", bufs=2))
        rbpool = ctx.enter_context(tc.tile_pool(name="rbpool", bufs=2))
        uspool = ctx.enter_context(tc.tile_pool(name="uspool", bufs=4))
        ypool = ctx.enter_context(tc.tile_pool(name="ypool", bufs=3))
        pspool = ctx.enter_context(tc.tile_pool(name="pspool", bufs=2, space="PSUM"))
        upool = ctx.enter_context(tc.tile_pool(name="upool", bufs=2, space="PSUM"))

        # ---- resident inputs -------------------------------------------------
        xt = const.tile([P, KC, N], BF16)       # x.T   (in-ch on partitions)
        wqk = const.tile([P, KC, 2 * CH], BF16)  # Wqk.T (in-ch on partitions)
        wv = const.tile([P, KC, CH], BF16)       # Wv.T
        wp = const.tile([P, PAIRS, DIM], BF16)   # Wproj.T slice (ch on part)
        bqk_sb = const.tile([1, 2 * CH], BF16)
        bv_sb = const.tile([1, CH], BF16)
        ones = const.tile([1, 512], BF16)
        v4 = const.tile([P, NKC, HPC * (HD + 1)], BF16)  # v + ones column

        # DMA order matters for warmup: the first QKV half only needs wqk
        # plus the first 512-col group of x.T, so land those first instead
        # of interleaving the full 5.5MB load ahead of the first matmul.
        for k in range(KC):
            nc.sync.dma_start(wqk[:, k, :], wqkT_d[k * P:(k + 1) * P, :])
        for nh4 in range(4):
            csl = slice(nh4 * 512, (nh4 + 1) * 512)
            for k in range(KC):
                nc.sync.dma_start(xt[:, k, csl], xT_d[k * P:(k + 1) * P, csl])
        for k in range(KC):
            nc.sync.dma_start(wv[:, k, :], wvT_d[k * P:(k + 1) * P, :])
        for t in range(PAIRS):
            nc.sync.dma_start(wp[:, t, :], wpT_d[t * P:(t + 1) * P, :])
        nc.sync.dma_start(bqk_sb[:], bqk_d[:])
        nc.sync.dma_start(bv_sb[:], bv_d[:])
        nc.vector.memset(ones[:], 1.0)
        # Dense memset to 1.0; the v drains below only overwrite columns
        # 0..63 of each 65-wide head block, leaving column 64 == 1.0 (the
        # fused softmax-rowsum column).
        nc.vector.memset(v4[:], 1.0)
        v4r = v4.rearrange("p n (h c) -> p n h c", c=HD + 1)

        qk_tiles = {}   # t -> (qt, kt)
        at_tiles = []

        def emit_qkv_half(t, part, nh):
            """One 512-col slice of pair t's q.T or k.T (nh in 0..3).
            Kept small (~2us of chained matmuls) so that when used as PE
            filler it never exceeds the Scalar engine's backlogged exp work
            — a 1024-wide part (~4us) starves the exp stream by ~2.5us.
            PSUM is allocated and drained within the call (a PSUM tile held
            across chunk boundaries deadlocks the in-order PE queue)."""
            if t not in qk_tiles:
                qt_ = qkpool.tile([P, N], BF16, tag="qk", name=f"qt{t}")
                kt_ = qkpool.tile([P, N], BF16, tag="qk", name=f"kt{t}")
                qk_tiles[t] = (qt_, kt_)
            qt_, kt_ = qk_tiles[t]
            colofs = t * P if part == "q" else CH + t * P
            dst = qt_ if part == "q" else kt_
            ps = pspool.tile([P, 512], F32, tag="s", name="qkps")
            xsl = slice(nh * 512, (nh + 1) * 512)
            for k in range(KC):
                nc.tensor.matmul(
                    ps[:],
                    lhsT=wqk[:, k, colofs:colofs + P],
                    rhs=xt[:, k, xsl],
                    start=(k == 0),
                    stop=(k == KC - 1 and not with_bias),
                )
            if with_bias:
                nc.tensor.matmul(
                    ps[:],
                    lhsT=bqk_sb[:, colofs:colofs + P],
                    rhs=ones[:, 0:512],
                    start=False, stop=True,
                )
            nc.vector.tensor_copy(dst[:, xsl], ps[:])

        def emit_v(s):
            """v for all 6 heads for sequence chunk s (with fused bias)."""
            vps = pspool.tile([P, CH], F32, tag="s", name="vps")
            for k in range(KC):
                nc.tensor.matmul(
                    vps[:],
                    lhsT=xt[:, k, s * P:(s + 1) * P],
                    rhs=wv[:, k, :],
                    start=(k == 0),
                    stop=(k == KC - 1 and not with_bias),
                )
            if with_bias:
                nc.tensor.matmul(
                    vps[:], lhsT=ones[:, 0:P], rhs=bv_sb[:],
                    start=False, stop=True,
                )
            nc.vector.tensor_copy(
                v4r[:, s, :, 0:HD],
                vps.rearrange("p (h c) -> p h c", c=HD),
            )

        def emit_proj_piece(z):
            """y.T piece: dims d*128..(d+1)*128, seq s4*512..(s4+1)*512.
            Transposed so the moving dim is 512 sequence positions (full
            bf16 stream rate; seq-major N=384 tiles pay the dependent-chain
            floor), and small enough (~1us) to ride as filler."""
            s4, dchunk = divmod(z, 6)
            yps = pspool.tile([P, 512], F32, tag="s", name="yps")
            for t in range(PAIRS):
                nc.tensor.matmul(
                    yps[:],
                    lhsT=wp[:, t, dchunk * P:(dchunk + 1) * P],
                    rhs=at_tiles[t][:, s4 * 512:(s4 + 1) * 512],
                    start=(t == 0), stop=(t == PAIRS - 1),
                )
            ysb = ypool.tile([P, 512], F32, tag="y", name="ysb")
            nc.vector.tensor_copy(ysb[:], yps[:])
            nc.sync.dma_start(
                y_d[dchunk * P:(dchunk + 1) * P, s4 * 512:(s4 + 1) * 512],
                ysb[:],
            )

        def emit_u(t, ua, ub, ea, eb, i):
            for n in range(QT // 512):
                nsl = slice(n * 512, (n + 1) * 512)
                nc.tensor.matmul(
                    ua[:, nsl], lhsT=v4r[:, i, 2 * t, :], rhs=ea[:, nsl],
                    start=(i == 0), stop=(i == NKC - 1),
                )
                nc.tensor.matmul(
                    ub[:, nsl], lhsT=v4r[:, i, 2 * t + 1, :], rhs=eb[:, nsl],
                    start=(i == 0), stop=(i == NKC - 1),
                )

        def drain_normalize(t, j, ua, ub, at):
            """Drain U psum to SBUF right away so the PSUM slots recycle
            fast, then normalize by the softmax row-sums (row HD) in
            512-wide halves so the DVE->GpSimd->DVE chain pipelines against
            its consumers instead of serializing ~10us at the kernel tail."""
            usa = uspool.tile([HD + 1, QT], F32, tag="us", name="usa")
            nc.vector.tensor_copy(usa[:], ua[:])
            usb = uspool.tile([HD + 1, QT], F32, tag="us", name="usb")
            nc.vector.tensor_copy(usb[:], ub[:])
            # normalize: out = U / rowsum  (rowsum in partition HD).
            # reciprocal_approx_fast (custom DVE op) corrupts data on HW
            # when its input sits at a non-zero base partition, so DMA the
            # rowsum row down to partition 0 first (engines can't shift
            # partitions; DMA can).
            for h in range(2):
                hsl = slice(h * 512, (h + 1) * 512)
                jhsl = slice(j * QT + h * 512, j * QT + (h + 1) * 512)
                rsa = rpool.tile([1, 512], F32, tag="rs", name="rsa")
                nc.sync.dma_start(rsa[:], usa[HD:HD + 1, hsl])
                ra = rpool.tile([1, 512], F32, tag="r", name="ra")
                nc.vector.reciprocal_approx_fast(ra[:], rsa[:])
                rba = rbpool.tile([HD, 512], F32, tag="rb", name="rba")
                nc.gpsimd.partition_broadcast(rba[:], ra[:])
                nc.vector.tensor_mul(at[0:HD, jhsl], usa[0:HD, hsl], rba[:])

                rsb = rpool.tile([1, 512], F32, tag="rs", name="rsb")
                nc.sync.dma_start(rsb[:], usb[HD:HD + 1, hsl])
                rb_ = rpool.tile([1, 512], F32, tag="r", name="rb_")
                nc.vector.reciprocal_approx_fast(rb_[:], rsb[:])
                rbb = rbpool.tile([HD, 512], F32, tag="rb", name="rbb")
                nc.gpsimd.partition_broadcast(rbb[:], rb_[:])
                # normalize head B into a bf16 staging tile, then DMA-shift
                # to partitions 64..127 (engines cannot shift partitions;
                # DMA can, and the dtype must already be bf16 since DMA
                # cannot convert)
                stg = uspool.tile([HD, 512], BF16, tag="stg", name="stg")
                nc.vector.tensor_mul(stg[:], usb[0:HD, hsl], rbb[:])
                nc.sync.dma_start(at[HD:P, jhsl], stg[:])

        # ---- pair 0 QKV + v (dense PE warm-up phase) ------------------------
        # Only what chunk (0,0,0) touches is computed up front: q cols
        # 0..1023 (the j=0 moving operand) and k-half 0 (key chunks 0..3).
        # k1/k2/k3 are first read at key chunks 4/8/12 and q2/q3 only in
        # j=1, so they ride as early fillers instead of serial warmup.
        emit_qkv_half(0, "q", 0)
        emit_qkv_half(0, "q", 1)
        emit_qkv_half(0, "k", 0)
        for s in range(NKC):
            emit_v(s)

        # filler schedule: (t, j, i) -> thunk emitted after that chunk's exp
        filler = {}
        for (prt0, nh0), i0 in zip(
            (("k", 1), ("k", 2), ("k", 3), ("q", 2), ("q", 3)),
            (0, 1, 3, 4, 6),
        ):
            filler[(0, 0, i0)] = (
                lambda prt0=prt0, nh0=nh0: emit_qkv_half(0, prt0, nh0)
            )
        for t_ in range(PAIRS - 1):
            pieces = [(prt, nh) for prt in ("q", "k") for nh in range(4)]
            spots = [(j_, i_) for j_ in range(2) for i_ in (2, 5, 8, 11)]
            for (prt, nh), (j_, i_) in zip(pieces, spots):
                filler[(t_, j_, i_)] = (
                    lambda prt=prt, nh=nh, t_=t_:
                    emit_qkv_half(t_ + 1, prt, nh)
                )
        for z in range(12):
            filler[(2, 1, 4 + z)] = lambda z=z: emit_proj_piece(z)

        # ---- attention: software-pipelined, continuous across j/pair
        # boundaries.  The U matmuls for chunk c are emitted AFTER chunk
        # c+1's S matmuls + exp, so the two K=64 row-group S matmuls (heads
        # A/B at array rows 0-63/64-127) keep queue priority and stay
        # adjacent — adjacent row-group pairs execute concurrently in the PE
        # array — and the pipeline never drains at a (t, j) boundary (the
        # per-boundary drain cost ~2.5us of scalar idle x 5 boundaries).
        pend = None   # (t, j, ua, ub, ea, eb, i, at)
        for t in range(PAIRS):
            qt_, kt_ = qk_tiles[t]
            at = atpool.tile([P, N], BF16, tag="at", name=f"at{t}")
            at_tiles.append(at)
            for j in range(N // QT):
                ua = upool.tile([HD + 1, QT], F32, tag="u", name="ua")
                ub = upool.tile([HD + 1, QT], F32, tag="u", name="ub")
                for i in range(NKC):
                    sa = pspool.tile([P, QT], F32, tag="s", name="sa")
                    sb = pspool.tile([P, QT], F32, tag="s", name="sb")
                    for n in range(QT // 512):
                        qsl = slice(j * QT + n * 512, j * QT + (n + 1) * 512)
                        nc.tensor.matmul(
                            sa[:, n * 512:(n + 1) * 512],
                            lhsT=kt_[0:HD, i * P:(i + 1) * P],
                            rhs=qt_[0:HD, qsl],
                            start=True, stop=True,
                        )
                        nc.tensor.matmul(
                            sb[:, n * 512:(n + 1) * 512],
                            lhsT=kt_[HD:P, i * P:(i + 1) * P],
                            rhs=qt_[HD:P, qsl],
                            start=True, stop=True,
                        )
                    ea = epool.tile([P, QT], BF16, tag="e", name="ea")
                    nc.scalar.activation(ea[:], sa[:], EXP, scale=SCALE)
                    eb = epool.tile([P, QT], BF16, tag="e", name="eb")
                    nc.scalar.activation(eb[:], sb[:], EXP, scale=SCALE)
                    if pend is not None:
                        pt, pj, pua, pub, pea, peb, pi, pat = pend
                        emit_u(pt, pua, pub, pea, peb, pi)
                        if pi == NKC - 1:
                            drain_normalize(pt, pj, pua, pub, pat)
                    pend = (t, j, ua, ub, ea, eb, i, at)
                    if (t, j, i) in filler:
                        filler[(t, j, i)]()
        pt, pj, pua, pub, pea, peb, pi, pat = pend
        emit_u(pt, pua, pub, pea, peb, pi)
        drain_normalize(pt, pj, pua, pub, pat)

        # ---- remaining projection (seq 1024..2047) --------------------------
        for z in range(12, 24):
            emit_proj_piece(z)


def build_program(with_bias=True):
    nc = bacc.Bacc(
        "TRN2", target_bir_lowering=False, debug=False, num_devices=NCORES
    )
    xT_d = nc.dram_tensor("xT", [DIM, N], BF16, kind="ExternalInput").ap()
    wqkT_d = nc.dram_tensor("wqkT", [DIM, 2 * CH], BF16, kind="ExternalInput").ap()
    wvT_d = nc.dram_tensor("wvT", [DIM, CH], BF16, kind="ExternalInput").ap()
    bqk_d = nc.dram_tensor("bqk", [1, 2 * CH], BF16, kind="ExternalInput").ap()
    bv_d = nc.dram_tensor("bv", [1, CH], BF16, kind="ExternalInput").ap()
    wpT_d = nc.dram_tensor("wpT", [CH, DIM], BF16, kind="ExternalInput").ap()
    # transposed output y.T [DIM, N]; the host transposes after gathering
    y_d = nc.dram_tensor("y", [DIM, N], F32, kind="ExternalOutput").ap()
    with tile.TileContext(nc) as tc:
        _emit(tc, xT_d, wqkT_d, wvT_d, bqk_d, bv_d, wpT_d, y_d, with_bias)
    nc.compile()
    return nc


def get_program(with_bias=True):
    if with_bias not in _PROGRAMS:
        _PROGRAMS[with_bias] = build_program(with_bias)
    return _PROGRAMS[with_bias]


def make_in_maps(x, Wqkv, bqkv, Wproj):
    import ml_dtypes

    bf16 = ml_dtypes.bfloat16
    x = np.ascontiguousarray(np.asarray(x, np.float32))
    Wqkv = np.asarray(Wqkv, np.float32)
    bqkv = np.asarray(bqkv, np.float32)
    in_maps = []
    for c in range(NCORES):
        b, g = divmod(c, GPB)
        cs = slice(g * CH, (g + 1) * CH)
        wq = Wqkv[0 * DIM:1 * DIM][cs]
        wk = Wqkv[1 * DIM:2 * DIM][cs]
        wv_ = Wqkv[2 * DIM:3 * DIM][cs]
        in_maps.append({
            "xT": np.ascontiguousarray(x[b].T).astype(bf16),
            "wqkT": np.ascontiguousarray(
                np.concatenate([wq, wk], 0).T).astype(bf16),
            "wvT": np.ascontiguousarray(wv_.T).astype(bf16),
            "bqk": np.concatenate(
                [bqkv[0 * DIM:1 * DIM][cs], bqkv[1 * DIM:2 * DIM][cs]]
            )[None, :].astype(bf16),
            "bv": bqkv[2 * DIM:3 * DIM][cs][None, :].astype(bf16),
            "wpT": np.ascontiguousarray(
                np.asarray(Wproj, np.float32)[:, cs].T).astype(bf16),
        })
    return in_maps


def combine_outputs(per_core_y, bproj):
    bproj = np.asarray(bproj, np.float32)
    out = np.empty((B, N, DIM), np.float32)
    for b in range(B):
        out[b] = (per_core_y[GPB * b] + per_core_y[GPB * b + 1]).T \
            + bproj[None, :]
    return out


def kernel(**inputs):
    ratio = int(np.asarray(inputs.get("ratio", 1)))
    assert ratio == 1, f"kernel specialized for ratio=1, got {ratio}"
    with_bias = bool(np.any(np.asarray(inputs["bqkv"], np.float32)))
    nc = get_program(with_bias)
    in_maps = make_in_maps(
        inputs["x"], inputs["Wqkv"], inputs["bqkv"], inputs["Wproj"]
    )
    res = run_bass_kernel_spmd(nc, in_maps, list(range(NCORES)))
    ys = [np.asarray(res.results[c]["y"], np.float32) for c in range(NCORES)]
    return combine_outputs(ys, inputs["bproj"])


# revision 64
# speedup vs baseline: 1.0040x; 1.0040x over previous
"""Trainium2 Bass kernel for nn_Attention_70557722739202.

Standard MHA block: qkv = x @ Wqkv.T + bqkv; attn = softmax(q k^T / 8);
out = (attn v) @ Wproj.T + bproj, with B=4, N=2048, C=768, H=12, hd=64
(ratio == 1 so the slimmable slicing is identity).

Sharding (8 cores): batch x head-group.  Core c handles batch c//2 and
heads [6*(c%2), 6*(c%2)+6).  Wqkv rows / Wproj cols are sharded by head;
each core emits a partial projection output [2048, 768] and the host sums
the two partials per batch (+ bproj).

Per-core dataflow (all PE matmuls in bfloat16, fp32 PSUM; bf16 moving
data streams 1 row/cycle at the PE's 2.4GHz vs 2 rows/cycle-pair for
4-byte fp32r, measured on HW):
  - x.T, W slices DMA'd with input-channel on partitions (bf16 host-cast).
  - q.T/k.T computed channel-major ([128, 2048] tiles, 2 heads per tile);
    v computed seq-major with a fused ones column for softmax row-sums.
    qkv biases are emitted as K=1 rank-1 matmuls only when nonzero.
  - S.T = k q^T per head via row-packed K=64 matmuls (2 heads concurrent
    in the PE array via tile_position row groups).  exp(S/8) runs on
    ScalarE straight out of PSUM with the 1/8 scale folded into the
    activation's free affine; no max subtraction (scores are O(1) by
    construction).  ScalarE exp is the pacing engine: 2 x [128,1024]
    activations per key chunk ~ 2.23us vs ~1.7us of PE work.
  - U.T = [v | 1]^T expS.T accumulated over key chunks -> rows 0..63 are
    the unnormalized attention output, row 64 the softmax denominator.
  - normalize: DVE reciprocal_approx_fast + GpSimd partition_broadcast +
    DVE multiply (head B through a bf16 staging tile + DMA partition
    shift).
  - projection: attnT chunks (stationary) x Wproj.T slices, PSUM K-accum.

Scheduling: software-pipelined per (pair, query-half): the U matmuls of
key chunk i are emitted after chunk i+1's S matmuls + exp; the exp
output pool is 6 deep so the Scalar engine can run a chunk ahead instead
of ping-ponging with the PE (with 2 bufs each exp waits for the U of the
previous chunk to release its slot - measured 1.1-1.3us stalls).  QKV for
pair t+1 and the first half of the projection are emitted as PE filler
inside pair t's attention.  Denser variants (continuous cross-boundary
pipeline with U deferred by 4 chunks) measured SLOWER: keeping the PE
gap-free saturates shared SBUF/PSUM bandwidth and inflates every
engine's per-instruction time by ~1.2x.
Measured on HW: fp32r baseline 429us -> bf16 369us -> this schedule 330us.
"""

import os
import sys

for _p in ("/opt/trn_rl_repo",):
    if os.path.isdir(_p) and _p not in sys.path:
        sys.path.insert(0, _p)

import numpy as np

import concourse.bacc as bacc
import concourse.mybir as mybir
import concourse.tile as tile
from concourse.bass_utils import run_bass_kernel_spmd

DIM = 768
NHEADS = 12
B, N = 4, 2048
HD = 64          # head dim
NCORES = 8
HPC = 6          # heads per core
PAIRS = 3        # head pairs per core
GPB = 2          # head groups per batch
CH = HPC * HD    # 384 output channels per core
SCALE = (DIM // NHEADS) ** -0.5
P = 128
QT = 1024        # query tile width (PSUM: 2 banks per S tile)
NKC = N // P     # 16 key chunks
KC = DIM // P    # 6 input-channel chunks
F32 = mybir.dt.float32
BF16 = mybir.dt.bfloat16
EXP = mybir.ActivationFunctionType.Exp

_PROGRAMS = {}


def _emit(tc, xT_d, wqkT_d, wvT_d, bqk_d, bv_d, wpT_d, y_d, with_bias=True):
    nc = tc.nc

    from contextlib import ExitStack

    with ExitStack() as ctx:
        const = ctx.enter_context(tc.tile_pool(name="const", bufs=1))
        qkpool = ctx.enter_context(tc.tile_pool(name="qkpool", bufs=4))
        atpool = ctx.enter_context(tc.tile_pool(name="atpool", bufs=3))
        # epool depth is the scalar-engine runahead: with only 2 bufs the
        # exp of chunk i+1 waits for U of chunk i to finish reading, so the
        # Scalar and Tensor engines ping-pong instead of streaming.  6 bufs
        # = 3 chunks in flight.
        epool = ctx.enter_context(tc.tile_pool(name="epool", bufs=6))
        rpool = ctx.enter_context(tc.tile_pool(name="rpool", bufs=2))
        rbpool = ctx.enter_context(tc.tile_pool(name="rbpool", bufs=2))
        uspool = ctx.enter_context(tc.tile_pool(name="uspool", bufs=4))
        ypool = ctx.enter_context(tc.tile_pool(name="ypool", bufs=3))
        pspool = ctx.enter_context(tc.tile_pool(name="pspool", bufs=2, space="PSUM"))
        upool = ctx.enter_context(tc.tile_pool(name="upool", bufs=2, space="PSUM"))

        # ---- resident inputs -------------------------------------------------
        xt = const.tile([P, KC, N], BF16)       # x.T   (in-ch on partitions)
        wqk = const.tile([P, KC, 2 * CH], BF16)  # Wqk.T (in-ch on partitions)
        wv = const.tile([P, KC, CH], BF16)       # Wv.T
        wp = const.tile([P, PAIRS, DIM], BF16)   # Wproj.T slice (ch on part)
        bqk_sb = const.tile([1, 2 * CH], BF16)
        bv_sb = const.tile([1, CH], BF16)
        ones = const.tile([1, 512], BF16)
        ones32 = const.tile([1, HD], F32)   # fp32 ones for tail PE-broadcast
        v4 = const.tile([P, NKC, HPC * (HD + 1)], BF16)  # v + ones column

        # DMA order matters for warmup: the first QKV half only needs wqk
        # plus the first 512-col group of x.T, so land those first instead
        # of interleaving the full 5.5MB load ahead of the first matmul.
        for k in range(KC):
            nc.sync.dma_start(wqk[:, k, :], wqkT_d[k * P:(k + 1) * P, :])
        for nh4 in range(4):
            csl = slice(nh4 * 512, (nh4 + 1) * 512)
            for k in range(KC):
                nc.sync.dma_start(xt[:, k, csl], xT_d[k * P:(k + 1) * P, csl])
        for k in range(KC):
            nc.sync.dma_start(wv[:, k, :], wvT_d[k * P:(k + 1) * P, :])
        for t in range(PAIRS):
            nc.sync.dma_start(wp[:, t, :], wpT_d[t * P:(t + 1) * P, :])
        nc.sync.dma_start(bqk_sb[:], bqk_d[:])
        nc.sync.dma_start(bv_sb[:], bv_d[:])
        nc.vector.memset(ones[:], 1.0)
        nc.vector.memset(ones32[:], 1.0)
        # Dense memset to 1.0; the v drains below only overwrite columns
        # 0..63 of each 65-wide head block, leaving column 64 == 1.0 (the
        # fused softmax-rowsum column).
        nc.vector.memset(v4[:], 1.0)
        v4r = v4.rearrange("p n (h c) -> p n h c", c=HD + 1)

        qk_tiles = {}   # t -> (qt, kt)
        at_tiles = []

        def emit_qkv_half(t, part, nh):
            """One 512-col slice of pair t's q.T or k.T (nh in 0..3).
            Kept small (~2us of chained matmuls) so that when used as PE
            filler it never exceeds the Scalar engine's backlogged exp work
            — a 1024-wide part (~4us) starves the exp stream by ~2.5us.
            PSUM is allocated and drained within the call (a PSUM tile held
            across chunk boundaries deadlocks the in-order PE queue)."""
            if t not in qk_tiles:
                qt_ = qkpool.tile([P, N], BF16, tag="qk", name=f"qt{t}")
                kt_ = qkpool.tile([P, N], BF16, tag="qk", name=f"kt{t}")
                qk_tiles[t] = (qt_, kt_)
            qt_, kt_ = qk_tiles[t]
            colofs = t * P if part == "q" else CH + t * P
            dst = qt_ if part == "q" else kt_
            ps = pspool.tile([P, 512], F32, tag="s", name="qkps")
            xsl = slice(nh * 512, (nh + 1) * 512)
            for k in range(KC):
                nc.tensor.matmul(
                    ps[:],
                    lhsT=wqk[:, k, colofs:colofs + P],
                    rhs=xt[:, k, xsl],
                    start=(k == 0),
                    stop=(k == KC - 1 and not with_bias),
                )
            if with_bias:
                nc.tensor.matmul(
                    ps[:],
                    lhsT=bqk_sb[:, colofs:colofs + P],
                    rhs=ones[:, 0:512],
                    start=False, stop=True,
                )
            nc.vector.tensor_copy(dst[:, xsl], ps[:])

        def emit_v(s):
            """v for all 6 heads for sequence chunk s (with fused bias)."""
            vps = pspool.tile([P, CH], F32, tag="s", name="vps")
            for k in range(KC):
                nc.tensor.matmul(
                    vps[:],
                    lhsT=xt[:, k, s * P:(s + 1) * P],
                    rhs=wv[:, k, :],
                    start=(k == 0),
                    stop=(k == KC - 1 and not with_bias),
                )
            if with_bias:
                nc.tensor.matmul(
                    vps[:], lhsT=ones[:, 0:P], rhs=bv_sb[:],
                    start=False, stop=True,
                )
            nc.vector.tensor_copy(
                v4r[:, s, :, 0:HD],
                vps.rearrange("p (h c) -> p h c", c=HD),
            )

        def emit_proj_piece(z):
            """y.T piece: dims d*128..(d+1)*128, seq s4*512..(s4+1)*512.
            Transposed so the moving dim is 512 sequence positions (full
            bf16 stream rate; seq-major N=384 tiles pay the dependent-chain
            floor), and small enough (~1us) to ride as filler."""
            s4, dchunk = divmod(z, 6)
            yps = pspool.tile([P, 512], F32, tag="s", name="yps")
            for t in range(PAIRS):
                nc.tensor.matmul(
                    yps[:],
                    lhsT=wp[:, t, dchunk * P:(dchunk + 1) * P],
                    rhs=at_tiles[t][:, s4 * 512:(s4 + 1) * 512],
                    start=(t == 0), stop=(t == PAIRS - 1),
                )
            ysb = ypool.tile([P, 512], F32, tag="y", name="ysb")
            nc.vector.tensor_copy(ysb[:], yps[:])
            nc.sync.dma_start(
                y_d[dchunk * P:(dchunk + 1) * P, s4 * 512:(s4 + 1) * 512],
                ysb[:],
            )

        def emit_u(t, ua, ub, ea, eb, i):
            for n in range(QT // 512):
                nsl = slice(n * 512, (n + 1) * 512)
                nc.tensor.matmul(
                    ua[:, nsl], lhsT=v4r[:, i, 2 * t, :], rhs=ea[:, nsl],
                    start=(i == 0), stop=(i == NKC - 1),
                )
                nc.tensor.matmul(
                    ub[:, nsl], lhsT=v4r[:, i, 2 * t + 1, :], rhs=eb[:, nsl],
                    start=(i == 0), stop=(i == NKC - 1),
                )

        def drain_normalize(t, j, ua, ub, at, pe_bcast=False):
            """Drain U psum to SBUF right away so the PSUM slots recycle
            fast, then normalize by the softmax row-sums (row HD) in
            512-wide halves so the DVE->GpSimd->DVE chain pipelines against
            its consumers instead of serializing ~10us at the kernel tail.
            pe_bcast (tail only): broadcast the reciprocal across partitions
            with a rank-1 fp32 PE matmul into a free PSUM slot instead of
            GpSimd — the PE and PSUM are idle in the tail and the matmul
            avoids a cross-engine queue hop on the critical path."""

            def bcast(r_):
                if pe_bcast:
                    rb_ps = pspool.tile([HD, 512], F32, tag="s", name="rbps")
                    nc.tensor.matmul(
                        rb_ps[:], lhsT=ones32[:], rhs=r_[:],
                        start=True, stop=True,
                    )
                    return rb_ps
                rb_sb = rbpool.tile([HD, 512], F32, tag="rb", name="rbsb")
                nc.gpsimd.partition_broadcast(rb_sb[:], r_[:])
                return rb_sb

            usa = uspool.tile([HD + 1, QT], F32, tag="us", name="usa")
            nc.vector.tensor_copy(usa[:], ua[:])
            usb = uspool.tile([HD + 1, QT], F32, tag="us", name="usb")
            nc.vector.tensor_copy(usb[:], ub[:])
            # normalize: out = U / rowsum  (rowsum in partition HD).
            # reciprocal_approx_fast (custom DVE op) corrupts data on HW
            # when its input sits at a non-zero base partition, so DMA the
            # rowsum row down to partition 0 first (engines can't shift
            # partitions; DMA can).
            for h in range(2):
                hsl = slice(h * 512, (h + 1) * 512)
                jhsl = slice(j * QT + h * 512, j * QT + (h + 1) * 512)
                rsa = rpool.tile([1, 512], F32, tag="rs", name="rsa")
                nc.sync.dma_start(rsa[:], usa[HD:HD + 1, hsl])
                ra = rpool.tile([1, 512], F32, tag="r", name="ra")
                nc.vector.reciprocal_approx_fast(ra[:], rsa[:])
                rba = bcast(ra)
                nc.vector.tensor_mul(at[0:HD, jhsl], usa[0:HD, hsl], rba[:])

                rsb = rpool.tile([1, 512], F32, tag="rs", name="rsb")
                nc.sync.dma_start(rsb[:], usb[HD:HD + 1, hsl])
                rb_ = rpool.tile([1, 512], F32, tag="r", name="rb_")
                nc.vector.reciprocal_approx_fast(rb_[:], rsb[:])
                rbb = bcast(rb_)
                # normalize head B into a bf16 staging tile, then DMA-shift
                # to partitions 64..127 (engines cannot shift partitions;
                # DMA can, and the dtype must already be bf16 since DMA
                # cannot convert)
                stg = uspool.tile([HD, 512], BF16, tag="stg", name="stg")
                nc.vector.tensor_mul(stg[:], usb[0:HD, hsl], rbb[:])
                nc.sync.dma_start(at[HD:P, jhsl], stg[:])

        # ---- pair 0 QKV + v (dense PE warm-up phase) ------------------------
        for part in ("q", "k"):
            for nh in range(4):
                emit_qkv_half(0, part, nh)
        for s in range(NKC):
            emit_v(s)

        # filler schedule: (t, j, i) -> thunk emitted after that chunk's exp
        filler = {}
        for t_ in range(PAIRS - 1):
            pieces = [(prt, nh) for prt in ("q", "k") for nh in range(4)]
            spots = [(j_, i_) for j_ in range(2) for i_ in (2, 5, 8, 11)]
            for (prt, nh), (j_, i_) in zip(pieces, spots):
                filler[(t_, j_, i_)] = (
                    lambda prt=prt, nh=nh, t_=t_:
                    emit_qkv_half(t_ + 1, prt, nh)
                )
        for z in range(12):
            filler[(2, 1, 4 + z)] = lambda z=z: emit_proj_piece(z)

        # ---- attention: software-pipelined, continuous across j/pair
        # boundaries.  The U matmuls for chunk c are emitted AFTER chunk
        # c+1's S matmuls + exp, so the two K=64 row-group S matmuls (heads
        # A/B at array rows 0-63/64-127) keep queue priority and stay
        # adjacent — adjacent row-group pairs execute concurrently in the PE
        # array — and the pipeline never drains at a (t, j) boundary (the
        # per-boundary drain cost ~2.5us of scalar idle x 5 boundaries).
        pend = None   # (t, j, ua, ub, ea, eb, i, at)
        for t in range(PAIRS):
            qt_, kt_ = qk_tiles[t]
            at = atpool.tile([P, N], BF16, tag="at", name=f"at{t}")
            at_tiles.append(at)
            for j in range(N // QT):
                ua = upool.tile([HD + 1, QT], F32, tag="u", name="ua")
                ub = upool.tile([HD + 1, QT], F32, tag="u", name="ub")
                for i in range(NKC):
                    sa = pspool.tile([P, QT], F32, tag="s", name="sa")
                    sb = pspool.tile([P, QT], F32, tag="s", name="sb")
                    for n in range(QT // 512):
                        qsl = slice(j * QT + n * 512, j * QT + (n + 1) * 512)
                        nc.tensor.matmul(
                            sa[:, n * 512:(n + 1) * 512],
                            lhsT=kt_[0:HD, i * P:(i + 1) * P],
                            rhs=qt_[0:HD, qsl],
                            start=True, stop=True,
                        )
                        nc.tensor.matmul(
                            sb[:, n * 512:(n + 1) * 512],
                            lhsT=kt_[HD:P, i * P:(i + 1) * P],
                            rhs=qt_[HD:P, qsl],
                            start=True, stop=True,
                        )
                    ea = epool.tile([P, QT], BF16, tag="e", name="ea")
                    nc.scalar.activation(ea[:], sa[:], EXP, scale=SCALE)
                    eb = epool.tile([P, QT], BF16, tag="e", name="eb")
                    nc.scalar.activation(eb[:], sb[:], EXP, scale=SCALE)
                    if pend is not None:
                        pt, pj, pua, pub, pea, peb, pi, pat = pend
                        emit_u(pt, pua, pub, pea, peb, pi)
                        if pi == NKC - 1:
                            drain_normalize(pt, pj, pua, pub, pat)
                    pend = (t, j, ua, ub, ea, eb, i, at)
                    if (t, j, i) in filler:
                        filler[(t, j, i)]()
        pt, pj, pua, pub, pea, peb, pi, pat = pend
        emit_u(pt, pua, pub, pea, peb, pi)
        drain_normalize(pt, pj, pua, pub, pat, pe_bcast=True)

        # ---- remaining projection (seq 1024..2047) --------------------------
        for z in range(12, 24):
            emit_proj_piece(z)


def build_program(with_bias=True):
    nc = bacc.Bacc(
        "TRN2", target_bir_lowering=False, debug=False, num_devices=NCORES
    )
    xT_d = nc.dram_tensor("xT", [DIM, N], BF16, kind="ExternalInput").ap()
    wqkT_d = nc.dram_tensor("wqkT", [DIM, 2 * CH], BF16, kind="ExternalInput").ap()
    wvT_d = nc.dram_tensor("wvT", [DIM, CH], BF16, kind="ExternalInput").ap()
    bqk_d = nc.dram_tensor("bqk", [1, 2 * CH], BF16, kind="ExternalInput").ap()
    bv_d = nc.dram_tensor("bv", [1, CH], BF16, kind="ExternalInput").ap()
    wpT_d = nc.dram_tensor("wpT", [CH, DIM], BF16, kind="ExternalInput").ap()
    # transposed output y.T [DIM, N]; the host transposes after gathering
    y_d = nc.dram_tensor("y", [DIM, N], F32, kind="ExternalOutput").ap()
    with tile.TileContext(nc) as tc:
        _emit(tc, xT_d, wqkT_d, wvT_d, bqk_d, bv_d, wpT_d, y_d, with_bias)
    nc.compile()
    return nc


def get_program(with_bias=True):
    if with_bias not in _PROGRAMS:
        _PROGRAMS[with_bias] = build_program(with_bias)
    return _PROGRAMS[with_bias]


def make_in_maps(x, Wqkv, bqkv, Wproj):
    import ml_dtypes

    bf16 = ml_dtypes.bfloat16
    x = np.ascontiguousarray(np.asarray(x, np.float32))
    Wqkv = np.asarray(Wqkv, np.float32)
    bqkv = np.asarray(bqkv, np.float32)
    in_maps = []
    for c in range(NCORES):
        b, g = divmod(c, GPB)
        cs = slice(g * CH, (g + 1) * CH)
        wq = Wqkv[0 * DIM:1 * DIM][cs]
        wk = Wqkv[1 * DIM:2 * DIM][cs]
        wv_ = Wqkv[2 * DIM:3 * DIM][cs]
        in_maps.append({
            "xT": np.ascontiguousarray(x[b].T).astype(bf16),
            "wqkT": np.ascontiguousarray(
                np.concatenate([wq, wk], 0).T).astype(bf16),
            "wvT": np.ascontiguousarray(wv_.T).astype(bf16),
            "bqk": np.concatenate(
                [bqkv[0 * DIM:1 * DIM][cs], bqkv[1 * DIM:2 * DIM][cs]]
            )[None, :].astype(bf16),
            "bv": bqkv[2 * DIM:3 * DIM][cs][None, :].astype(bf16),
            "wpT": np.ascontiguousarray(
                np.asarray(Wproj, np.float32)[:, cs].T).astype(bf16),
        })
    return in_maps


def combine_outputs(per_core_y, bproj):
    bproj = np.asarray(bproj, np.float32)
    out = np.empty((B, N, DIM), np.float32)
    for b in range(B):
        out[b] = (per_core_y[GPB * b] + per_core_y[GPB * b + 1]).T \
            + bproj[None, :]
    return out


def kernel(**inputs):
    ratio = int(np.asarray(inputs.get("ratio", 1)))
    assert ratio == 1, f"kernel specialized for ratio=1, got {ratio}"
    with_bias = bool(np.any(np.asarray(inputs["bqkv"], np.float32)))
    nc = get_program(with_bias)
    in_maps = make_in_maps(
        inputs["x"], inputs["Wqkv"], inputs["bqkv"], inputs["Wproj"]
    )
    res = run_bass_kernel_spmd(nc, in_maps, list(range(NCORES)))
    ys = [np.asarray(res.results[c]["y"], np.float32) for c in range(NCORES)]
    return combine_outputs(ys, inputs["bproj"])


# revision 65
# speedup vs baseline: 1.0101x; 1.0061x over previous
"""Trainium2 Bass kernel for nn_Attention_70557722739202.

Standard MHA block: qkv = x @ Wqkv.T + bqkv; attn = softmax(q k^T / 8);
out = (attn v) @ Wproj.T + bproj, with B=4, N=2048, C=768, H=12, hd=64
(ratio == 1 so the slimmable slicing is identity).

Sharding (8 cores): batch x head-group.  Core c handles batch c//2 and
heads [6*(c%2), 6*(c%2)+6).  Wqkv rows / Wproj cols are sharded by head;
each core emits a partial projection output [2048, 768] and the host sums
the two partials per batch (+ bproj).

Per-core dataflow (all PE matmuls in bfloat16, fp32 PSUM; bf16 moving
data streams 1 row/cycle at the PE's 2.4GHz vs 2 rows/cycle-pair for
4-byte fp32r, measured on HW):
  - x.T, W slices DMA'd with input-channel on partitions (bf16 host-cast).
  - q.T/k.T computed channel-major ([128, 2048] tiles, 2 heads per tile);
    v computed seq-major with a fused ones column for softmax row-sums.
    qkv biases are emitted as K=1 rank-1 matmuls only when nonzero.
  - S.T = k q^T per head via row-packed K=64 matmuls (2 heads concurrent
    in the PE array via tile_position row groups).  exp(S/8) runs on
    ScalarE straight out of PSUM with the 1/8 scale folded into the
    activation's free affine; no max subtraction (scores are O(1) by
    construction).  ScalarE exp is the pacing engine: 2 x [128,1024]
    activations per key chunk ~ 2.23us vs ~1.7us of PE work.
  - U.T = [v | 1]^T expS.T accumulated over key chunks -> rows 0..63 are
    the unnormalized attention output, row 64 the softmax denominator.
  - normalize: DVE reciprocal_approx_fast + GpSimd partition_broadcast +
    DVE multiply (head B through a bf16 staging tile + DMA partition
    shift).
  - projection: attnT chunks (stationary) x Wproj.T slices, PSUM K-accum.

Scheduling: software-pipelined per (pair, query-half): the U matmuls of
key chunk i are emitted after chunk i+1's S matmuls + exp; the exp
output pool is 6 deep so the Scalar engine can run a chunk ahead instead
of ping-ponging with the PE (with 2 bufs each exp waits for the U of the
previous chunk to release its slot - measured 1.1-1.3us stalls).  QKV for
pair t+1 and the first half of the projection are emitted as PE filler
inside pair t's attention.  Denser variants (continuous cross-boundary
pipeline with U deferred by 4 chunks) measured SLOWER: keeping the PE
gap-free saturates shared SBUF/PSUM bandwidth and inflates every
engine's per-instruction time by ~1.2x.
Measured on HW: fp32r baseline 429us -> bf16 369us -> this schedule 330us.
"""

import os
import sys

for _p in ("/opt/trn_rl_repo",):
    if os.path.isdir(_p) and _p not in sys.path:
        sys.path.insert(0, _p)

import numpy as np

import concourse.bacc as bacc
import concourse.mybir as mybir
import concourse.tile as tile
from concourse.bass_utils import run_bass_kernel_spmd

DIM = 768
NHEADS = 12
B, N = 4, 2048
HD = 64          # head dim
NCORES = 8
HPC = 6          # heads per core
PAIRS = 3        # head pairs per core
GPB = 2          # head groups per batch
CH = HPC * HD    # 384 output channels per core
SCALE = (DIM // NHEADS) ** -0.5
P = 128
QT = 1024        # query tile width (PSUM: 2 banks per S tile)
NKC = N // P     # 16 key chunks
KC = DIM // P    # 6 input-channel chunks
F32 = mybir.dt.float32
BF16 = mybir.dt.bfloat16
EXP = mybir.ActivationFunctionType.Exp

_PROGRAMS = {}


def _emit(tc, xT_d, wqkT_d, wvT_d, bqk_d, bv_d, wpT_d, y_d, with_bias=True):
    nc = tc.nc

    from contextlib import ExitStack

    with ExitStack() as ctx:
        const = ctx.enter_context(tc.tile_pool(name="const", bufs=1))
        qkpool = ctx.enter_context(tc.tile_pool(name="qkpool", bufs=4))
        atpool = ctx.enter_context(tc.tile_pool(name="atpool", bufs=3))
        # epool depth is the scalar-engine runahead: with only 2 bufs the
        # exp of chunk i+1 waits for U of chunk i to finish reading, so the
        # Scalar and Tensor engines ping-pong instead of streaming.  6 bufs
        # = 3 chunks in flight.
        epool = ctx.enter_context(tc.tile_pool(name="epool", bufs=6))
        rpool = ctx.enter_context(tc.tile_pool(name="rpool", bufs=2))
        rbpool = ctx.enter_context(tc.tile_pool(name="rbpool", bufs=2))
        uspool = ctx.enter_context(tc.tile_pool(name="uspool", bufs=4))
        ypool = ctx.enter_context(tc.tile_pool(name="ypool", bufs=3))
        pspool = ctx.enter_context(tc.tile_pool(name="pspool", bufs=2, space="PSUM"))
        upool = ctx.enter_context(tc.tile_pool(name="upool", bufs=2, space="PSUM"))

        # ---- resident inputs -------------------------------------------------
        xt = const.tile([P, KC, N], BF16)       # x.T   (in-ch on partitions)
        wqk = const.tile([P, KC, 2 * CH], BF16)  # Wqk.T (in-ch on partitions)
        wv = const.tile([P, KC, CH], BF16)       # Wv.T
        wp = const.tile([P, PAIRS, DIM], BF16)   # Wproj.T slice (ch on part)
        bqk_sb = const.tile([1, 2 * CH], BF16)
        bv_sb = const.tile([1, CH], BF16)
        ones = const.tile([1, 512], BF16)
        v4 = const.tile([P, NKC, HPC * (HD + 1)], BF16)  # v + ones column

        # DMA order matters for warmup: the first QKV half only needs wqk
        # plus the first 512-col group of x.T, so land those first instead
        # of interleaving the full 5.5MB load ahead of the first matmul.
        for k in range(KC):
            nc.sync.dma_start(wqk[:, k, :], wqkT_d[k * P:(k + 1) * P, :])
        for nh4 in range(4):
            csl = slice(nh4 * 512, (nh4 + 1) * 512)
            for k in range(KC):
                nc.sync.dma_start(xt[:, k, csl], xT_d[k * P:(k + 1) * P, csl])
        for k in range(KC):
            nc.sync.dma_start(wv[:, k, :], wvT_d[k * P:(k + 1) * P, :])
        for t in range(PAIRS):
            nc.sync.dma_start(wp[:, t, :], wpT_d[t * P:(t + 1) * P, :])
        nc.sync.dma_start(bqk_sb[:], bqk_d[:])
        nc.sync.dma_start(bv_sb[:], bv_d[:])
        nc.vector.memset(ones[:], 1.0)
        # Dense memset to 1.0; the v drains below only overwrite columns
        # 0..63 of each 65-wide head block, leaving column 64 == 1.0 (the
        # fused softmax-rowsum column).
        nc.vector.memset(v4[:], 1.0)
        v4r = v4.rearrange("p n (h c) -> p n h c", c=HD + 1)

        qk_tiles = {}   # t -> (qt, kt)
        at_tiles = []

        def emit_qkv_half(t, part, nh):
            """One 512-col slice of pair t's q.T or k.T (nh in 0..3).
            Kept small (~2us of chained matmuls) so that when used as PE
            filler it never exceeds the Scalar engine's backlogged exp work
            — a 1024-wide part (~4us) starves the exp stream by ~2.5us.
            PSUM is allocated and drained within the call (a PSUM tile held
            across chunk boundaries deadlocks the in-order PE queue)."""
            if t not in qk_tiles:
                qt_ = qkpool.tile([P, N], BF16, tag="qk", name=f"qt{t}")
                kt_ = qkpool.tile([P, N], BF16, tag="qk", name=f"kt{t}")
                qk_tiles[t] = (qt_, kt_)
            qt_, kt_ = qk_tiles[t]
            colofs = t * P if part == "q" else CH + t * P
            dst = qt_ if part == "q" else kt_
            ps = pspool.tile([P, 512], F32, tag="s", name="qkps")
            xsl = slice(nh * 512, (nh + 1) * 512)
            for k in range(KC):
                nc.tensor.matmul(
                    ps[:],
                    lhsT=wqk[:, k, colofs:colofs + P],
                    rhs=xt[:, k, xsl],
                    start=(k == 0),
                    stop=(k == KC - 1 and not with_bias),
                )
            if with_bias:
                nc.tensor.matmul(
                    ps[:],
                    lhsT=bqk_sb[:, colofs:colofs + P],
                    rhs=ones[:, 0:512],
                    start=False, stop=True,
                )
            nc.vector.tensor_copy(dst[:, xsl], ps[:])

        def emit_v(s):
            """v for all 6 heads for sequence chunk s (with fused bias)."""
            vps = pspool.tile([P, CH], F32, tag="s", name="vps")
            for k in range(KC):
                nc.tensor.matmul(
                    vps[:],
                    lhsT=xt[:, k, s * P:(s + 1) * P],
                    rhs=wv[:, k, :],
                    start=(k == 0),
                    stop=(k == KC - 1 and not with_bias),
                )
            if with_bias:
                nc.tensor.matmul(
                    vps[:], lhsT=ones[:, 0:P], rhs=bv_sb[:],
                    start=False, stop=True,
                )
            nc.vector.tensor_copy(
                v4r[:, s, :, 0:HD],
                vps.rearrange("p (h c) -> p h c", c=HD),
            )

        def emit_proj_piece(z):
            """y.T piece: dims d*128..(d+1)*128, seq s4*512..(s4+1)*512.
            Transposed so the moving dim is 512 sequence positions (full
            bf16 stream rate; seq-major N=384 tiles pay the dependent-chain
            floor), and small enough (~1us) to ride as filler."""
            s4, dchunk = divmod(z, 6)
            yps = pspool.tile([P, 512], F32, tag="s", name="yps")
            for t in range(PAIRS):
                nc.tensor.matmul(
                    yps[:],
                    lhsT=wp[:, t, dchunk * P:(dchunk + 1) * P],
                    rhs=at_tiles[t][:, s4 * 512:(s4 + 1) * 512],
                    start=(t == 0), stop=(t == PAIRS - 1),
                )
            ysb = ypool.tile([P, 512], F32, tag="y", name="ysb")
            nc.vector.tensor_copy(ysb[:], yps[:])
            nc.sync.dma_start(
                y_d[dchunk * P:(dchunk + 1) * P, s4 * 512:(s4 + 1) * 512],
                ysb[:],
            )

        def emit_u(t, ua, ub, ea, eb, i):
            for n in range(QT // 512):
                nsl = slice(n * 512, (n + 1) * 512)
                nc.tensor.matmul(
                    ua[:, nsl], lhsT=v4r[:, i, 2 * t, :], rhs=ea[:, nsl],
                    start=(i == 0), stop=(i == NKC - 1),
                )
                nc.tensor.matmul(
                    ub[:, nsl], lhsT=v4r[:, i, 2 * t + 1, :], rhs=eb[:, nsl],
                    start=(i == 0), stop=(i == NKC - 1),
                )

        def drain_normalize(t, j, ua, ub, at):
            """Drain U psum to SBUF right away so the PSUM slots recycle
            fast, then normalize by the softmax row-sums (row HD) in
            512-wide halves so the DVE->GpSimd->DVE chain pipelines against
            its consumers instead of serializing ~10us at the kernel tail."""
            usa = uspool.tile([HD + 1, QT], F32, tag="us", name="usa")
            nc.vector.tensor_copy(usa[:], ua[:])
            usb = uspool.tile([HD + 1, QT], F32, tag="us", name="usb")
            nc.vector.tensor_copy(usb[:], ub[:])
            # normalize: out = U / rowsum  (rowsum in partition HD).
            # reciprocal_approx_fast (custom DVE op) corrupts data on HW
            # when its input sits at a non-zero base partition, so DMA the
            # rowsum row down to partition 0 first (engines can't shift
            # partitions; DMA can).
            for h in range(2):
                hsl = slice(h * 512, (h + 1) * 512)
                jhsl = slice(j * QT + h * 512, j * QT + (h + 1) * 512)
                rsa = rpool.tile([1, 512], F32, tag="rs", name="rsa")
                nc.sync.dma_start(rsa[:], usa[HD:HD + 1, hsl])
                ra = rpool.tile([1, 512], F32, tag="r", name="ra")
                nc.vector.reciprocal_approx_fast(ra[:], rsa[:])
                rba = rbpool.tile([HD, 512], F32, tag="rb", name="rba")
                nc.gpsimd.partition_broadcast(rba[:], ra[:])
                nc.vector.tensor_mul(at[0:HD, jhsl], usa[0:HD, hsl], rba[:])

                rsb = rpool.tile([1, 512], F32, tag="rs", name="rsb")
                nc.sync.dma_start(rsb[:], usb[HD:HD + 1, hsl])
                rb_ = rpool.tile([1, 512], F32, tag="r", name="rb_")
                nc.vector.reciprocal_approx_fast(rb_[:], rsb[:])
                rbb = rbpool.tile([HD, 512], F32, tag="rb", name="rbb")
                nc.gpsimd.partition_broadcast(rbb[:], rb_[:])
                # normalize head B into a bf16 staging tile, then DMA-shift
                # to partitions 64..127 (engines cannot shift partitions;
                # DMA can, and the dtype must already be bf16 since DMA
                # cannot convert)
                stg = uspool.tile([HD, 512], BF16, tag="stg", name="stg")
                nc.vector.tensor_mul(stg[:], usb[0:HD, hsl], rbb[:])
                nc.sync.dma_start(at[HD:P, jhsl], stg[:])

        # ---- pair 0 QKV + v (dense PE warm-up phase) ------------------------
        for part in ("q", "k"):
            for nh in range(4):
                emit_qkv_half(0, part, nh)
        for s in range(NKC):
            emit_v(s)

        # filler schedule: (t, j, i) -> thunk emitted after that chunk's exp
        filler = {}
        for t_ in range(PAIRS - 1):
            pieces = [(prt, nh) for prt in ("q", "k") for nh in range(4)]
            spots = [(j_, i_) for j_ in range(2) for i_ in (2, 5, 8, 11)]
            for (prt, nh), (j_, i_) in zip(pieces, spots):
                filler[(t_, j_, i_)] = (
                    lambda prt=prt, nh=nh, t_=t_:
                    emit_qkv_half(t_ + 1, prt, nh)
                )
        for z in range(12):
            filler[(2, 1, 4 + z)] = lambda z=z: emit_proj_piece(z)

        # ---- attention: software-pipelined, continuous across j/pair
        # boundaries.  The U matmuls for chunk c are emitted AFTER chunk
        # c+1's S matmuls + exp, so the two K=64 row-group S matmuls (heads
        # A/B at array rows 0-63/64-127) keep queue priority and stay
        # adjacent — adjacent row-group pairs execute concurrently in the PE
        # array — and the pipeline never drains at a (t, j) boundary (the
        # per-boundary drain cost ~2.5us of scalar idle x 5 boundaries).
        pend = None   # (t, j, ua, ub, ea, eb, i, at)
        for t in range(PAIRS):
            qt_, kt_ = qk_tiles[t]
            at = atpool.tile([P, N], BF16, tag="at", name=f"at{t}")
            at_tiles.append(at)
            for j in range(N // QT):
                ua = upool.tile([HD + 1, QT], F32, tag="u", name="ua")
                ub = upool.tile([HD + 1, QT], F32, tag="u", name="ub")
                for i in range(NKC):
                    sa = pspool.tile([P, QT], F32, tag="s", name="sa")
                    sb = pspool.tile([P, QT], F32, tag="s", name="sb")
                    for n in range(QT // 512):
                        qsl = slice(j * QT + n * 512, j * QT + (n + 1) * 512)
                        nc.tensor.matmul(
                            sa[:, n * 512:(n + 1) * 512],
                            lhsT=kt_[0:HD, i * P:(i + 1) * P],
                            rhs=qt_[0:HD, qsl],
                            start=True, stop=True,
                        )
                        nc.tensor.matmul(
                            sb[:, n * 512:(n + 1) * 512],
                            lhsT=kt_[HD:P, i * P:(i + 1) * P],
                            rhs=qt_[HD:P, qsl],
                            start=True, stop=True,
                        )
                    ea = epool.tile([P, QT], BF16, tag="e", name="ea")
                    nc.scalar.activation(ea[:], sa[:], EXP, scale=SCALE)
                    eb = epool.tile([P, QT], BF16, tag="e", name="eb")
                    nc.scalar.activation(eb[:], sb[:], EXP, scale=SCALE)
                    if pend is not None:
                        pt, pj, pua, pub, pea, peb, pi, pat = pend
                        emit_u(pt, pua, pub, pea, peb, pi)
                        if pi == NKC - 1:
                            drain_normalize(pt, pj, pua, pub, pat)
                    pend = (t, j, ua, ub, ea, eb, i, at)
                    if (t, j, i) in filler:
                        filler[(t, j, i)]()
        pt, pj, pua, pub, pea, peb, pi, pat = pend
        emit_u(pt, pua, pub, pea, peb, pi)
        drain_normalize(pt, pj, pua, pub, pat)

        # ---- remaining projection (seq 1024..2047) --------------------------
        for z in range(12, 24):
            emit_proj_piece(z)


def build_program(with_bias=True):
    nc = bacc.Bacc(
        "TRN2", target_bir_lowering=False, debug=False, num_devices=NCORES
    )
    xT_d = nc.dram_tensor("xT", [DIM, N], BF16, kind="ExternalInput").ap()
    wqkT_d = nc.dram_tensor("wqkT", [DIM, 2 * CH], BF16, kind="ExternalInput").ap()
    wvT_d = nc.dram_tensor("wvT", [DIM, CH], BF16, kind="ExternalInput").ap()
    bqk_d = nc.dram_tensor("bqk", [1, 2 * CH], BF16, kind="ExternalInput").ap()
    bv_d = nc.dram_tensor("bv", [1, CH], BF16, kind="ExternalInput").ap()
    wpT_d = nc.dram_tensor("wpT", [CH, DIM], BF16, kind="ExternalInput").ap()
    # transposed output y.T [DIM, N]; the host transposes after gathering
    y_d = nc.dram_tensor("y", [DIM, N], F32, kind="ExternalOutput").ap()
    with tile.TileContext(nc) as tc:
        _emit(tc, xT_d, wqkT_d, wvT_d, bqk_d, bv_d, wpT_d, y_d, with_bias)
    nc.compile()
    return nc


def get_program(with_bias=True):
    if with_bias not in _PROGRAMS:
        _PROGRAMS[with_bias] = build_program(with_bias)
    return _PROGRAMS[with_bias]


def make_in_maps(x, Wqkv, bqkv, Wproj):
    import ml_dtypes

    bf16 = ml_dtypes.bfloat16
    x = np.ascontiguousarray(np.asarray(x, np.float32))
    Wqkv = np.asarray(Wqkv, np.float32)
    bqkv = np.asarray(bqkv, np.float32)
    in_maps = []
    for c in range(NCORES):
        b, g = divmod(c, GPB)
        cs = slice(g * CH, (g + 1) * CH)
        wq = Wqkv[0 * DIM:1 * DIM][cs]
        wk = Wqkv[1 * DIM:2 * DIM][cs]
        wv_ = Wqkv[2 * DIM:3 * DIM][cs]
        in_maps.append({
            "xT": np.ascontiguousarray(x[b].T).astype(bf16),
            "wqkT": np.ascontiguousarray(
                np.concatenate([wq, wk], 0).T).astype(bf16),
            "wvT": np.ascontiguousarray(wv_.T).astype(bf16),
            "bqk": np.concatenate(
                [bqkv[0 * DIM:1 * DIM][cs], bqkv[1 * DIM:2 * DIM][cs]]
            )[None, :].astype(bf16),
            "bv": bqkv[2 * DIM:3 * DIM][cs][None, :].astype(bf16),
            "wpT": np.ascontiguousarray(
                np.asarray(Wproj, np.float32)[:, cs].T).astype(bf16),
        })
    return in_maps


def combine_outputs(per_core_y, bproj):
    bproj = np.asarray(bproj, np.float32)
    out = np.empty((B, N, DIM), np.float32)
    for b in range(B):
        out[b] = (per_core_y[GPB * b] + per_core_y[GPB * b + 1]).T \
            + bproj[None, :]
    return out


def kernel(**inputs):
    ratio = int(np.asarray(inputs.get("ratio", 1)))
    assert ratio == 1, f"kernel specialized for ratio=1, got {ratio}"
    with_bias = bool(np.any(np.asarray(inputs["bqkv"], np.float32)))
    nc = get_program(with_bias)
    in_maps = make_in_maps(
        inputs["x"], inputs["Wqkv"], inputs["bqkv"], inputs["Wproj"]
    )
    res = run_bass_kernel_spmd(nc, in_maps, list(range(NCORES)))
    ys = [np.asarray(res.results[c]["y"], np.float32) for c in range(NCORES)]
    return combine_outputs(ys, inputs["bproj"])
